# revision 23
# baseline (speedup 1.0000x reference)
"""CRF Viterbi decode (B=64, S=512, D=768, T=64) on 8 trn2 cores.

Sharding: data-parallel over batch (8 batch items per core). Each core:
  Phase 1: emissions em[b,t,j] = x[b,t,:] @ W[j,:]  (PE matmul, K=768 in 6 chunks)
  Phase 2: Viterbi forward max-plus scan over t (stores score history, no bp)
  Phase 3: backtrack recomputing argmax per step via one-hot PE matmuls + max8

Key layout trick: scores kept as scoreT[j, b] (tag-partitioned). Per forward
step the PE builds s[j,(b,i)] = trans_eff[i,j] + score[b,i] + em_t[b,j] with
three accumulating matmuls (identity x transT, all-ones x masked-score M,
identity x em-broadcast); the DVE does one masked-mult (M = scoreT * I) and
one max-reduce per step. No DMA or row flatten in the loop. Accumulation
order (trans+score)+em reproduces the reference's f32 rounding bitwise.
trans_eff[i,j] = transitions[i,j] + bias[j] (bias folded in, exact).
Backtrack runs as two independent batch-group chains with hardware
max/max_index (first-occurrence, matching jnp.argmax).
"""

import sys

sys.path.insert(0, "/opt/trn_rl_repo")

import numpy as np

import concourse.bass as bass
from concourse import bacc
import concourse.mybir as mybir
import concourse.tile as tile
from concourse.masks import make_identity

F32 = mybir.dt.float32
U16 = mybir.dt.uint16
I32 = mybir.dt.int32

B, S, D, T = 64, 512, 768, 64
NCORES = 8
BL = B // NCORES  # 8 batch items per core
KC = D // 128  # 6 contraction chunks


def build_nc(s_steps=S, groups=2, fwd_only=False):
    nc = bacc.Bacc(None, target_bir_lowering=False)
    x_d = nc.dram_tensor("xt", [BL, D, s_steps], F32, kind="ExternalInput")
    wt_d = nc.dram_tensor("wt", [D, T], F32, kind="ExternalInput")
    transT_d = nc.dram_tensor("transT", [T, T], F32, kind="ExternalInput")
    bias_d = nc.dram_tensor("biasc", [T, 1], F32, kind="ExternalInput")
    tags_d = nc.dram_tensor("tags", [BL, s_steps], I32, kind="ExternalOutput")

    with tile.TileContext(nc) as tc:
        with (
            tc.tile_pool(name="persist", bufs=1) as pp,
            tc.tile_pool(name="work", bufs=3) as wp,
            tc.tile_pool(name="psum", bufs=2 if groups <= 2 else 1, space="PSUM") as psp,
            tc.tile_pool(name="psum1", bufs=2 if groups <= 2 else 1, space="PSUM") as ps1,
        ):
            # ---- constants ----
            id64 = pp.tile([T, T], F32, tag="id64")
            make_identity(nc, id64[:])
            ones1 = pp.tile([1, T], F32, tag="ones1")
            nc.gpsimd.memset(ones1[:], 1.0)
            ones64 = pp.tile([T, T], F32, tag="ones64")
            nc.gpsimd.memset(ones64[:], 1.0)
            iota_row = pp.tile([BL, T], U16, tag="iota_row")
            nc.gpsimd.iota(iota_row[:], pattern=[[1, T]], base=0, channel_multiplier=0)

            transT = pp.tile([T, T], F32, tag="transT")
            nc.sync.dma_start(transT[:], transT_d[:])
            bias = pp.tile([T, 1], F32, tag="bias")
            nc.sync.dma_start(bias[:], bias_d[:])
            wt = pp.tile([128, KC, T], F32, tag="wt")
            nc.sync.dma_start(wt[:], wt_d[:].rearrange("(c p) j -> p c j", p=128))

            # ---- persistent state ----
            emT = pp.tile([T, s_steps, BL], F32, tag="emT")  # em[b,t,j] at [j,t,b]
            hist = pp.tile([T, s_steps, BL], F32, tag="hist")  # scoreT history
            bph0 = pp.tile([BL // 2, s_steps, 8], U16, tag="bph0")
            bph1 = pp.tile([BL // 2, s_steps, 8], U16, tag="bph1")
            bphs = [bph0, bph1]  # chosen tags per batch-group (col 0)

            # ---- phase 1: emissions ----
            for b in range(BL):
                xT = wp.tile([128, KC, s_steps], F32, tag="xT")
                nc.sync.dma_start(
                    xT[:], x_d[b].rearrange("(c p) t -> p c t", p=128)
                )
                pe = psp.tile([T, s_steps], F32, tag="ps0")
                for c in range(KC):
                    nc.tensor.matmul(
                        pe[:],
                        wt[:, c, :],
                        xT[:, c, :],
                        start=(c == 0),
                        stop=(c == KC - 1),
                    )
                # emT[:, :, b] = pe  (strided write, scalar engine keeps DVE free)
                nc.scalar.activation(
                    emT[:, :, b], pe[:], mybir.ActivationFunctionType.Copy
                )

            # ---- phase 2: forward scan (2 pipelined batch-groups) ----
            # em folded into the PE accumulation: max_i(score+trans) + em
            # == max_i(score + trans + em) since em is constant over i.
            # The next-step score row [1, (b,i)] is built on the PE:
            # M = scoreT (bcast over i) * I (bcast over b), then a
            # ones-column contraction sums the single nonzero per (b,i).
            G = groups
            BG = BL // G  # 4 batch items per group
            id64_bg = id64[:, None, :].broadcast_to((T, BG, T))
            Ms = [None] * G

            def make_M(g, t, bs):
                # M[k,(b,i)] = score_t[b,k]*I[k,i]; the next step contracts it
                # with an all-ones lhsT: sum_k M[k,(b,i)] = score_t[b,i],
                # broadcast across all output partitions in one matmul.
                M = wp.tile([T, BG, T], F32, tag=f"M{g}")
                nc.vector.tensor_tensor(
                    M[:],
                    hist[:, t, bs][:, :, None].broadcast_to((T, BG, T)),
                    id64_bg,
                    op=mybir.AluOpType.mult,
                )
                Ms[g] = M

            for g in range(G):
                bs = slice(g * BG, (g + 1) * BG)
                nc.vector.tensor_add(
                    hist[:, 0, bs], emT[:, 0, bs], bias[:].broadcast_to((T, BG))
                )
                make_M(g, 0, bs)

            transT_bg = transT[:, None, :].broadcast_to((T, BG, T))
            for t in range(1, s_steps):
                for g in range(G):
                    bs = slice(g * BG, (g + 1) * BG)
                    ps = psp.tile([T, BG * T], F32, tag=f"ps{g}")
                    ps3 = ps[:].rearrange("j (b i) -> j b i", b=BG)
                    # accumulation order (trans + score) + em matches the
                    # reference's rounding exactly (f32 add is commutative)
                    nc.tensor.matmul(ps3, id64[:], transT_bg, start=True, stop=False)
                    nc.tensor.matmul(
                        ps[:],
                        ones64[:],
                        Ms[g][:].rearrange("j b i -> j (b i)"),
                        start=False,
                        stop=False,
                    )
                    nc.tensor.matmul(
                        ps3,
                        id64[:],
                        emT[:, t, bs][:, :, None].broadcast_to((T, BG, T)),
                        start=False,
                        stop=True,
                    )
                    nc.vector.tensor_reduce(
                        hist[:, t, bs],
                        ps3,
                        axis=mybir.AxisListType.X,
                        op=mybir.AluOpType.max,
                    )
                    if t < s_steps - 1:
                        make_M(g, t, bs)

            # ---- phase 3: backtrack (2 independent batch-group chains) ----
            # No bp stored in forward: per step recompute
            # argmax_i(score_t[b,i] + trans_eff[i, j*_b]) via one-hot matmuls.
            # mm order (score + trans) matches the reference rounding exactly.
            GB = BL // 2
            oTs = [None, None]
            for g in range(2):
                bs = slice(g * GB, (g + 1) * GB)
                ptb = ps1.tile([GB, T], F32, tag=f"pt{g}")
                nc.tensor.transpose(ptb[:], hist[:, s_steps - 1, bs], id64[:])
                sb = wp.tile([GB, T], F32, tag=f"sb{g}")
                nc.scalar.activation(sb[:], ptb[:], mybir.ActivationFunctionType.Copy)
                m8 = wp.tile([GB, 8], F32, tag=f"m8{g}")
                nc.vector.max(m8[:], sb[:])
                nc.vector.max_index(bphs[g][:, s_steps - 1, :], m8[:], sb[:])
                obT = wp.tile([GB, T], F32, tag=f"obT{g}")
                nc.vector.tensor_tensor(
                    obT[:],
                    iota_row[:GB, :],
                    bphs[g][:, s_steps - 1, 0:1].broadcast_to((GB, T)),
                    op=mybir.AluOpType.is_equal,
                )
                po = psp.tile([T, GB], F32, tag=f"ps{g}")
                nc.tensor.transpose(po[:], obT[:], id64[:GB, :GB])
                oT = wp.tile([T, GB], F32, tag=f"oT{g}")
                nc.scalar.activation(oT[:], po[:], mybir.ActivationFunctionType.Copy)
                oTs[g] = oT

            for t in range(s_steps - 2, -1, -1):
                for g in range(2):
                    bs = slice(g * GB, (g + 1) * GB)
                    pb = ps1.tile([GB, T], F32, tag=f"pt{g}")
                    nc.tensor.matmul(
                        pb[:], hist[:, t, bs], id64[:], start=True, stop=False
                    )
                    nc.tensor.matmul(
                        pb[:], oTs[g][:], transT[:], start=False, stop=True
                    )
                    sb = wp.tile([GB, T], F32, tag=f"sb{g}")
                    nc.scalar.activation(
                        sb[:], pb[:], mybir.ActivationFunctionType.Copy
                    )
                    m8 = wp.tile([GB, 8], F32, tag=f"m8{g}")
                    nc.vector.max(m8[:], sb[:])
                    nc.vector.max_index(bphs[g][:, t, :], m8[:], sb[:])
                    if t > 0:
                        obT = wp.tile([GB, T], F32, tag=f"obT{g}")
                        nc.vector.tensor_tensor(
                            obT[:],
                            iota_row[:GB, :],
                            bphs[g][:, t, 0:1].broadcast_to((GB, T)),
                            op=mybir.AluOpType.is_equal,
                        )
                        po = psp.tile([T, GB], F32, tag=f"ps{g}")
                        nc.tensor.transpose(po[:], obT[:], id64[:GB, :GB])
                        oT = wp.tile([T, GB], F32, tag=f"oT{g}")
                        nc.scalar.activation(
                            oT[:], po[:], mybir.ActivationFunctionType.Copy
                        )
                        oTs[g] = oT

            # ---- output ----
            for g in range(2):
                GB2 = BL // 2
                tags_sb = pp.tile([GB2, s_steps], I32, tag=f"tags_sb{g}")
                nc.vector.tensor_copy(tags_sb[:], bphs[g][:, :, 0])
                nc.sync.dma_start(tags_d[g * GB2 : (g + 1) * GB2, :], tags_sb[:])

    nc.compile()
    return nc


_NC_CACHE = {}


def kernel(x, W, b, transitions):
    from concourse.bass_utils import run_bass_kernel_spmd

    x = np.asarray(x, dtype=np.float32)
    xt = np.ascontiguousarray(x.transpose(0, 2, 1))  # [B, D, S]
    W = np.asarray(W, dtype=np.float32)
    b = np.asarray(b, dtype=np.float32)
    trans = np.asarray(transitions, dtype=np.float32)

    wt = np.ascontiguousarray(W.T)  # [D, T]
    transT = np.ascontiguousarray(trans.T + b[:, None])  # [j, i] = trans[i,j]+b[j]
    biasc = np.ascontiguousarray(b[:, None])  # [T, 1]

    if "nc" not in _NC_CACHE:
        _NC_CACHE["nc"] = build_nc(S)
    nc = _NC_CACHE["nc"]

    in_maps = []
    for c in range(NCORES):
        in_maps.append(
            {
                "xt": np.ascontiguousarray(xt[c * BL : (c + 1) * BL]),
                "wt": wt,
                "transT": transT,
                "biasc": biasc,
            }
        )
    res = run_bass_kernel_spmd(nc, in_maps, core_ids=list(range(NCORES)))
    out = np.concatenate([r["tags"] for r in res.results], axis=0)
    return out.astype(np.int32)


if __name__ == "__main__":
    rng = np.random.default_rng(0)
    x = rng.standard_normal((B, S, D), dtype=np.float32)
    W = rng.standard_normal((T, D), dtype=np.float32) * 0.02
    b = np.zeros((T,), dtype=np.float32)
    trans = rng.uniform(-0.1, 0.1, size=(T, T)).astype(np.float32)
    out = kernel(x=x, W=W, b=b, transitions=trans)
    print(out.shape, out.dtype, out[:2, :8])
